# revision 6
# baseline (speedup 1.0000x reference)
"""MultiHeadAttention Trainium2 Bass kernel (v2: interleaved pipeline).

Problem: B=8, H=W=32 (S=1024), C=512, 8 heads x 64 dim.
Sharding: data-parallel over batch, one batch element per NeuronCore (8 cores).

v2 design vs v1 (150us baseline):
- x converted to f16 (gpsimd) so input PE-transposes run at 1 cycle/col
  (fp32 transposes are 2 cycles/col with a serial weight load).
- Projections and attention all in fp16 (10 mantissa bits): x/W converted
  once, Q/K use fp16 W stationary with fp16 xT moving; V uses fp16 xT
  stationary (FWL fast weight load) with fp16 W moving.
- V_aug stationary padded to 128 cols (64 d + ones + zeros) so attV weight
  loads use FWL and hide under the previous matmul.
- Single-phase emission: projections/transposes/evacuations are interleaved
  into the attention stream as filler so the PE never idles waiting on exp
  (ACT) and never cools (HAM throttle).
- Per-head-pair output DMAs issued as soon as each slice is evacuated.

Precision: fp16 operands, fp32 accumulation. Simulated end-to-end absmax
rel err ~1.3e-3 (threshold 2e-2).
"""
import sys

import numpy as np

if "/opt/trn_rl_repo" not in sys.path:
    sys.path.insert(0, "/opt/trn_rl_repo")

import concourse.bacc as bacc
import concourse.mybir as mybir
import concourse.tile as tile
from concourse import masks
from concourse.bass_utils import run_bass_kernel_spmd

B, HS, WS, C = 8, 32, 32, 512
S = HS * WS          # 1024
D = 512
HEADS = 8
HD = 64              # head dim
N_CORES = 8

f32 = mybir.dt.float32
f16 = mybir.dt.float16
Exp = mybir.ActivationFunctionType.Exp


def build_nc():
    nc = bacc.Bacc("TRN2", target_bir_lowering=False, debug=False,
                   num_devices=N_CORES)

    x_d = {}
    w_d = {}
    b_d = {}
    for name in ("q", "k", "v"):
        x_d[name] = nc.dram_tensor(f"{name}_in", [S, C], f32, kind="ExternalInput")
        w_d[name] = nc.dram_tensor(f"W{name}", [C, D], f32, kind="ExternalInput")
        b_d[name] = nc.dram_tensor(f"b{name}", [D], f32, kind="ExternalInput")
    out_d = nc.dram_tensor("out", [S, D], f32, kind="ExternalOutput")

    with tile.TileContext(nc) as tc:
        with (
            tc.tile_pool(name="const", bufs=1) as cpool,
            tc.tile_pool(name="x32", bufs=5) as x32_pool,
            tc.tile_pool(name="xb", bufs=3) as xb_pool,
            tc.tile_pool(name="xT", bufs=1) as xt_pool,
            tc.tile_pool(name="wbuf", bufs=1) as w_pool,
            tc.tile_pool(name="proj", bufs=1) as proj_pool,
            tc.tile_pool(name="att", bufs=3) as att_pool,
            tc.tile_pool(name="ot", bufs=4) as ot_pool,
            tc.tile_pool(name="ps_p", bufs=2, space="PSUM") as ps_p,
            tc.tile_pool(name="ps_s", bufs=2, space="PSUM") as ps_s,
            tc.tile_pool(name="ps_o", bufs=2, space="PSUM") as ps_o,
        ):
            # ---------------- constants ----------------
            ident_f16 = cpool.tile([128, 128], f16)
            masks.make_identity(nc, ident_f16[:])
            ident_f32 = cpool.tile([128, 128], f32)
            masks.make_identity(nc, ident_f32[:])
            ones_sb = cpool.tile([128, 512], f32)
            nc.vector.memset(ones_sb[:], 1.0)
            ones_16 = cpool.tile([1, 512], f16)
            nc.vector.tensor_copy(ones_16[:], ones_sb[0:1, :])

            # ---------------- persistent tensors ----------------
            QT = proj_pool.tile([128, 4, S], f16, name="QT")  # [d%128, d//128, s]
            KT = proj_pool.tile([128, 4, S], f16, name="KT")
            # V_aug padded: [s%128, s//128, head, 128]; col 64 = 1.0 (denom),
            # cols 65.. = 0 so the 128-wide stationary gets FWL.
            V = proj_pool.tile([128, 8, HEADS, 128], f16, name="V")
            zz = cpool.tile([128, 512], f16)
            nc.vector.memset(zz[:], 0.0)
            for st8 in range(8):
                nc.vector.tensor_copy(
                    V[:, st8, :, HD + 1:],
                    zz[:, 0:8 * 63].rearrange("p (a o) -> p a o", a=8))
            nc.vector.tensor_copy(
                V[:, :, :, HD:HD + 1],
                ones_sb[:, 0:64].rearrange("p (a b o) -> p a b o", a=8, b=8))
            o_stage = proj_pool.tile([128, 8, D], f32, name="o_stage")

            # xT tiles (f16): [c%128, c//128, s]
            xT = {}
            for name in ("q", "k", "v"):
                xT[name] = xt_pool.tile([128, 4, S], f16, name=f"xT_{name}",
                                        tag=f"xT_{name}")

            # ---------------- weight/bias DMAs (priority order) ------------
            w_sb = {}
            b_sb = {}
            for name in ("k", "q", "v"):
                w_sb[name] = w_pool.tile([128, 4, D], f32, name=f"w_{name}",
                                         tag=f"w_{name}")
                nc.sync.dma_start(
                    w_sb[name][:],
                    w_d[name][:].rearrange("(cc p) d -> p cc d", p=128))
            for name in ("k", "q"):
                b_sb[name] = w_pool.tile([128, 4], f32, name=f"b_{name}",
                                         tag=f"b_{name}")
                nc.sync.dma_start(
                    b_sb[name][:], b_d[name][:].rearrange("(dt p) -> p dt", p=128))
            bv_sb = w_pool.tile([1, D], f32, name="bv_sb", tag="bv_sb")
            nc.sync.dma_start(
                bv_sb[:], b_d["v"][:].rearrange("(o d) -> o d", o=1))
            bv_16 = w_pool.tile([1, D], f16, name="bv_16", tag="bv_16")
            nc.vector.tensor_copy(bv_16[:], bv_sb[:])
            # fp16 weights (converted once on gpsimd)
            w16 = {}
            for name in ("k", "q", "v"):
                w16[name] = w_pool.tile([128, 4, D], f16, name=f"w16_{name}",
                                        tag=f"w16_{name}")
                nc.gpsimd.tensor_copy(w16[name][:], w_sb[name][:])

            # ---------------- x DMAs (chunked, priority order) --------------
            # chunk ch covers t = 2ch, 2ch+1 (rows 256ch .. 256ch+255)
            x32 = {}   # (name, ch) -> tile
            def dma_x(name, ch):
                t_ = x32_pool.tile([128, 2, C], f32, name=f"x32_{name}{ch}",
                                   tag="x32")
                x_r = x_d[name][:].rearrange("(t p) c -> p t c", p=128)
                nc.sync.dma_start(t_[:], x_r[:, 2 * ch:2 * ch + 2, :])
                x32[(name, ch)] = t_

            for name, ch in (("k", 0), ("k", 1), ("k", 2), ("k", 3),
                             ("q", 0), ("q", 1), ("v", 0), ("v", 1),
                             ("q", 2), ("q", 3), ("v", 2), ("v", 3)):
                dma_x(name, ch)

            # ---------------- emission helpers ----------------
            xb = {}

            def cv(name, ch):
                """Convert x chunk to f16 on gpsimd."""
                t_ = xb_pool.tile([128, 2, C], f16, name=f"xb_{name}{ch}",
                                  tag="xb")
                nc.gpsimd.tensor_copy(t_[:], x32[(name, ch)][:])
                xb[(name, ch)] = t_

            def T_t(name, t):
                """PE-transpose x rows t*128..t*128+128 into xT[name]."""
                src = xb[(name, t // 2)]
                ti = t % 2
                pst = ps_p.tile([128, 4, 128], f16, tag="pp",
                                name=f"pst_{name}_{t}")
                for cc in range(4):
                    nc.tensor.transpose(
                        pst[:, cc, :],
                        src[:, ti, cc * 128:(cc + 1) * 128],
                        ident_f16[:])
                nc.vector.tensor_copy(
                    xT[name][:, :, t * 128:(t + 1) * 128], pst[:])

            def projqk(name, dt, qhs=(0, 1)):
                """Q/K projection d-chunk dt for the given q-halves.

                stationary: fp16 W slice, moving: fp16 xT.
                """
                tgt = QT if name == "q" else KT
                psqs = {}
                for qh in qhs:
                    psqs[qh] = ps_p.tile([128, 512], f32, tag="pp",
                                         name=f"psq_{name}{dt}_{qh}")
                for cc in range(4):
                    w_slice = w16[name][:, cc, dt * 128:(dt + 1) * 128]
                    for qh in qhs:
                        nc.tensor.matmul(
                            psqs[qh][:],
                            w_slice,
                            xT[name][:, cc, qh * 512:(qh + 1) * 512],
                            start=(cc == 0), stop=(cc == 3))
                for qh in qhs:
                    nc.vector.tensor_scalar_add(
                        tgt[:, dt, qh * 512:(qh + 1) * 512], psqs[qh][:],
                        b_sb[name][:, dt:dt + 1])

            def projv(st):
                """V projection s-chunk st: stationary xT_v fp16 (FWL),
                moving W fp16; ones-column bias matmul; evac to V_aug."""
                psv = ps_p.tile([128, 512], f32, tag="pp", name=f"psv_{st}")
                for cc in range(4):
                    nc.tensor.matmul(
                        psv[:],
                        xT["v"][:, cc, st * 128:(st + 1) * 128],
                        w16["v"][:, cc, :],
                        start=(cc == 0), stop=False)
                nc.tensor.matmul(
                    psv[:], ones_16[0:1, 0:128], bv_16[0:1, :],
                    start=False, stop=True)
                nc.vector.tensor_copy(
                    V[:, st, :, 0:HD],
                    psv[:].rearrange("p (h e) -> p h e", h=HEADS))

            # ---------------- filler queue ----------------
            fillers = []

            def fill(n=1):
                for _ in range(n):
                    if fillers:
                        fillers.pop(0)()

            def drain_fillers():
                while fillers:
                    fillers.pop(0)()

            # ---------------- attention ----------------
            # Deferred tail (attV kt=7 + evacuation) per sub-block, flushed
            # inside the next sub-block after its first exp.
            pend = [None]

            def scores_exp(hp, qh, kt):
                heads = (2 * hp, 2 * hp + 1)
                pss = ps_s.tile([128, 2, 512], f32, tag="pss",
                                name=f"pss_{hp}_{qh}_{kt}")
                for i, h in enumerate(heads):
                    po = (h % 2) * HD
                    nc.tensor.matmul(
                        pss[:, i, :],
                        KT[po:po + HD, hp, kt * 128:(kt + 1) * 128],
                        QT[po:po + HD, hp, qh * 512:(qh + 1) * 512],
                        start=True, stop=True)
                attT = att_pool.tile([128, 2, 512], f16, tag="attT",
                                     name=f"attT_{hp}_{qh}_{kt}")
                nc.scalar.activation(attT[:], pss[:], Exp, scale=0.125)
                return attT

            def attention(hp):
                heads = (2 * hp, 2 * hp + 1)
                for qh in range(2):
                    atts = []
                    atts.append(scores_exp(hp, qh, 0))
                    if pend[0] is not None:
                        pend[0]()
                        pend[0] = None
                    pso = {}
                    for h in heads:
                        pso[h] = ps_o.tile([128, 512], f32,
                                           name=f"pso{h}_{qh}", tag="pso")
                    # software pipeline: attv(kt-1) emitted after scores(kt)
                    # so the PE never waits on the exp(kt) it just fed.
                    for kt in range(1, 8):
                        atts.append(scores_exp(hp, qh, kt))
                        fill(1)
                        for i, h in enumerate(heads):
                            nc.tensor.matmul(
                                pso[h][:],
                                V[:, kt - 1, h, :],
                                atts[kt - 1][:, i, :],
                                start=(kt == 1), stop=False)

                    att7 = atts[7]

                    def tail(heads=heads, pso=pso, att7=att7, qh=qh):
                        for i, h in enumerate(heads):
                            nc.tensor.matmul(
                                pso[h][:], V[:, 7, h, :], att7[:, i, :],
                                start=False, stop=True)
                        for h in heads:
                            oT = ot_pool.tile([HD + 1, 512], f32, tag="oT")
                            nc.vector.tensor_copy(oT[:], pso[h][0:HD + 1, :])
                            pbt = ps_o.tile([128, 4, HD + 1], f32, tag="pso",
                                            name=f"pbt{h}_{qh}")
                            for qs in range(4):
                                nc.tensor.transpose(
                                    pbt[:, qs, :],
                                    oT[:, qs * 128:(qs + 1) * 128],
                                    ident_f32[0:HD + 1, 0:HD + 1])
                            rec = ot_pool.tile([128, 4], f32, tag="rec")
                            nc.vector.reciprocal(rec[:], pbt[:, :, HD])
                            for qs in range(4):
                                qt = qh * 4 + qs
                                nc.vector.tensor_scalar_mul(
                                    o_stage[:, qt, h * HD:(h + 1) * HD],
                                    pbt[:, qs, 0:HD],
                                    rec[:, qs:qs + 1])

                    pend[0] = tail

            # ---------------- prefix ----------------
            for ch in range(4):
                cv("k", ch)
                T_t("k", 2 * ch)
                T_t("k", 2 * ch + 1)
            projqk("k", 0)
            for ch in range(2):
                cv("q", ch)
                T_t("q", 2 * ch)
                T_t("q", 2 * ch + 1)
            projqk("q", 0, qhs=(0,))
            cv("v", 0)
            T_t("v", 0)
            projv(0)
            T_t("v", 1)
            projv(1)

            # ---------------- filler schedule ----------------
            def u(*fns):
                def unit():
                    for f in fns:
                        f()
                return unit

            fillers.extend([
                # during (0, qh0): keep V st chain >= 1 ahead of attv, plus
                # finish xT_q and QT dt0 qh1 (needed at (0, qh1) kt0).
                u(lambda: cv("v", 1), lambda: T_t("v", 2), lambda: projv(2)),
                u(lambda: T_t("v", 3), lambda: projv(3),
                  lambda: cv("q", 2), lambda: T_t("q", 4), lambda: T_t("q", 5)),
                u(lambda: cv("v", 2), lambda: T_t("v", 4), lambda: projv(4),
                  lambda: cv("q", 3), lambda: T_t("q", 6), lambda: T_t("q", 7)),
                u(lambda: T_t("v", 5), lambda: projv(5),
                  lambda: projqk("q", 0, qhs=(1,))),
                u(lambda: cv("v", 3), lambda: T_t("v", 6), lambda: projv(6)),
                u(lambda: T_t("v", 7), lambda: projv(7)),
                u(lambda: projqk("k", 1)),
                # during (0, qh1)
                u(lambda: projqk("q", 1)),
                u(lambda: projqk("k", 2)),
                u(lambda: projqk("q", 2)),
                u(lambda: projqk("k", 3)),
                # during (1, qh0)
                u(lambda: projqk("q", 3)),
            ])

            out_r = out_d[:].rearrange("(t p) d -> p t d", p=128)

            attention(0)
            attention(1)
            # hp0 tails flushed inside attention(1)'s first sub-block
            nc.sync.dma_start(out_r[:, :, 0:128], o_stage[:, :, 0:128])
            attention(2)
            nc.sync.dma_start(out_r[:, :, 128:256], o_stage[:, :, 128:256])
            attention(3)
            nc.sync.dma_start(out_r[:, :, 256:384], o_stage[:, :, 256:384])
            pend[0]()
            pend[0] = None
            drain_fillers()
            nc.sync.dma_start(out_r[:, :, 384:512], o_stage[:, :, 384:512])

    nc.compile()
    return nc


_NC = None


def _get_nc():
    global _NC
    if _NC is None:
        _NC = build_nc()
    return _NC


def _make_in_maps(inputs):
    in_maps = []
    for b in range(B):
        m = {
            "q_in": np.ascontiguousarray(np.asarray(inputs["q_in"])[b].reshape(S, C)),
            "k_in": np.ascontiguousarray(np.asarray(inputs["k_in"])[b].reshape(S, C)),
            "v_in": np.ascontiguousarray(np.asarray(inputs["v_in"])[b].reshape(S, C)),
            "Wq": np.asarray(inputs["Wq"]), "bq": np.asarray(inputs["bq"]),
            "Wk": np.asarray(inputs["Wk"]), "bk": np.asarray(inputs["bk"]),
            "Wv": np.asarray(inputs["Wv"]), "bv": np.asarray(inputs["bv"]),
        }
        in_maps.append(m)
    return in_maps


def kernel(**inputs):
    nc = _get_nc()
    res = run_bass_kernel_spmd(nc, _make_in_maps(inputs), list(range(N_CORES)))
    out = np.stack([res.results[i]["out"] for i in range(B)])
    return out.reshape(B, HS, WS, D).astype(np.float32)


if __name__ == "__main__":
    rng = np.random.default_rng(0)
    ins = {
        "q_in": rng.standard_normal((B, HS, WS, C), dtype=np.float32),
        "k_in": rng.standard_normal((B, HS, WS, C), dtype=np.float32),
        "v_in": rng.standard_normal((B, HS, WS, C), dtype=np.float32),
        "Wq": (rng.standard_normal((C, D)) / np.sqrt(C)).astype(np.float32),
        "Wk": (rng.standard_normal((C, D)) / np.sqrt(C)).astype(np.float32),
        "Wv": (rng.standard_normal((C, D)) / np.sqrt(C)).astype(np.float32),
        "bq": np.zeros(D, np.float32),
        "bk": np.zeros(D, np.float32),
        "bv": np.zeros(D, np.float32),
    }
    out = kernel(**ins)
    print("out shape:", out.shape, "finite:", np.isfinite(out).all())


# revision 7
# speedup vs baseline: 1.1407x; 1.1407x over previous
"""MultiHeadAttention Trainium2 Bass kernel (v2: interleaved pipeline).

Problem: B=8, H=W=32 (S=1024), C=512, 8 heads x 64 dim.
Sharding: data-parallel over batch, one batch element per NeuronCore (8 cores).

v2 design vs v1 (150us baseline):
- x converted to fp16 (DVE) so input PE-transposes run at 1 cycle/col
  (fp32 transposes are 2 cycles/col with a serial weight load).
- Projections and attention all in fp16 (10 mantissa bits): x/W converted
  once, Q/K use fp16 W stationary with fp16 xT moving; V uses fp16 xT
  stationary (FWL fast weight load) with fp16 W moving.
- V_aug stationary padded to 128 cols (64 d + ones + zeros) so attV weight
  loads use FWL and hide under the previous matmul.
- Single-phase emission: projections/transposes/evacuations are interleaved
  into the attention stream as filler so the PE never idles waiting on exp
  (ACT) and never cools (HAM throttle).
- Per-head-pair output DMAs issued as soon as each slice is evacuated.

Precision: fp16 operands, fp32 accumulation. Simulated end-to-end absmax
rel err ~1.3e-3 (threshold 2e-2).
"""
import sys

import numpy as np

if "/opt/trn_rl_repo" not in sys.path:
    sys.path.insert(0, "/opt/trn_rl_repo")

import concourse.bacc as bacc
import concourse.mybir as mybir
import concourse.tile as tile
from concourse import masks
from concourse.bass_utils import run_bass_kernel_spmd

B, HS, WS, C = 8, 32, 32, 512
S = HS * WS          # 1024
D = 512
HEADS = 8
HD = 64              # head dim
N_CORES = 8

f32 = mybir.dt.float32
f16 = mybir.dt.float16
Exp = mybir.ActivationFunctionType.Exp


def build_nc():
    nc = bacc.Bacc("TRN2", target_bir_lowering=False, debug=False,
                   num_devices=N_CORES)

    x_d = {}
    w_d = {}
    b_d = {}
    for name in ("q", "k", "v"):
        x_d[name] = nc.dram_tensor(f"{name}_in", [S, C], f32, kind="ExternalInput")
        w_d[name] = nc.dram_tensor(f"W{name}", [C, D], f32, kind="ExternalInput")
        b_d[name] = nc.dram_tensor(f"b{name}", [D], f32, kind="ExternalInput")
    out_d = nc.dram_tensor("out", [S, D], f32, kind="ExternalOutput")

    with tile.TileContext(nc) as tc:
        with (
            tc.tile_pool(name="const", bufs=1) as cpool,
            tc.tile_pool(name="x32", bufs=5) as x32_pool,
            tc.tile_pool(name="xb", bufs=3) as xb_pool,
            tc.tile_pool(name="xT", bufs=1) as xt_pool,
            tc.tile_pool(name="wbuf", bufs=1) as w_pool,
            tc.tile_pool(name="proj", bufs=1) as proj_pool,
            tc.tile_pool(name="att", bufs=3) as att_pool,
            tc.tile_pool(name="ot", bufs=4) as ot_pool,
            tc.tile_pool(name="ps_p", bufs=2, space="PSUM") as ps_p,
            tc.tile_pool(name="ps_s", bufs=2, space="PSUM") as ps_s,
            tc.tile_pool(name="ps_o", bufs=2, space="PSUM") as ps_o,
        ):
            # ---------------- constants ----------------
            ident_f16 = cpool.tile([128, 128], f16)
            masks.make_identity(nc, ident_f16[:])
            ident_f32 = cpool.tile([128, 128], f32)
            masks.make_identity(nc, ident_f32[:])
            ones_sb = cpool.tile([128, 512], f32)
            nc.vector.memset(ones_sb[:], 1.0)
            ones_16 = cpool.tile([1, 512], f16)
            nc.vector.tensor_copy(ones_16[:], ones_sb[0:1, :])

            # ---------------- persistent tensors ----------------
            QT = proj_pool.tile([128, 4, S], f16, name="QT")  # [d%128, d//128, s]
            KT = proj_pool.tile([128, 4, S], f16, name="KT")
            # V_aug padded: [s%128, s//128, head, 128]; col 64 = 1.0 (denom),
            # cols 65.. = 0 so the 128-wide stationary gets FWL.
            V = proj_pool.tile([128, 8, HEADS, 128], f16, name="V")
            zz = cpool.tile([128, 512], f16)
            nc.vector.memset(zz[:], 0.0)
            for st8 in range(8):
                nc.vector.tensor_copy(
                    V[:, st8, :, HD + 1:],
                    zz[:, 0:8 * 63].rearrange("p (a o) -> p a o", a=8))
            nc.vector.tensor_copy(
                V[:, :, :, HD:HD + 1],
                ones_sb[:, 0:64].rearrange("p (a b o) -> p a b o", a=8, b=8))
            o_stage = proj_pool.tile([128, 8, D], f32, name="o_stage")

            # xT tiles (f16): [c%128, c//128, s]
            xT = {}
            for name in ("q", "k", "v"):
                xT[name] = xt_pool.tile([128, 4, S], f16, name=f"xT_{name}",
                                        tag=f"xT_{name}")

            # ---------------- weight/bias DMAs (priority order) ------------
            w_sb = {}
            b_sb = {}
            for name in ("k", "q", "v"):
                w_sb[name] = w_pool.tile([128, 4, D], f32, name=f"w_{name}",
                                         tag=f"w_{name}")
                nc.sync.dma_start(
                    w_sb[name][:],
                    w_d[name][:].rearrange("(cc p) d -> p cc d", p=128))
            for name in ("k", "q"):
                b_sb[name] = w_pool.tile([128, 4], f32, name=f"b_{name}",
                                         tag=f"b_{name}")
                nc.sync.dma_start(
                    b_sb[name][:], b_d[name][:].rearrange("(dt p) -> p dt", p=128))
            bv_sb = w_pool.tile([1, D], f32, name="bv_sb", tag="bv_sb")
            nc.sync.dma_start(
                bv_sb[:], b_d["v"][:].rearrange("(o d) -> o d", o=1))
            bv_16 = w_pool.tile([1, D], f16, name="bv_16", tag="bv_16")
            nc.vector.tensor_copy(bv_16[:], bv_sb[:])
            # fp16 weights (converted once on DVE)
            w16 = {}
            for name in ("k", "q", "v"):
                w16[name] = w_pool.tile([128, 4, D], f16, name=f"w16_{name}",
                                        tag=f"w16_{name}")
                nc.vector.tensor_copy(w16[name][:], w_sb[name][:])

            # ---------------- x DMAs (chunked, priority order) --------------
            # chunk ch covers t = 2ch, 2ch+1 (rows 256ch .. 256ch+255)
            x32 = {}   # (name, ch) -> tile
            def dma_x(name, ch):
                t_ = x32_pool.tile([128, 2, C], f32, name=f"x32_{name}{ch}",
                                   tag="x32")
                x_r = x_d[name][:].rearrange("(t p) c -> p t c", p=128)
                nc.sync.dma_start(t_[:], x_r[:, 2 * ch:2 * ch + 2, :])
                x32[(name, ch)] = t_

            for name, ch in (("k", 0), ("k", 1), ("k", 2), ("k", 3),
                             ("q", 0), ("q", 1), ("v", 0), ("v", 1),
                             ("q", 2), ("q", 3), ("v", 2), ("v", 3)):
                dma_x(name, ch)

            # ---------------- emission helpers ----------------
            xb = {}

            def cv(name, ch):
                """Convert x chunk to fp16 on DVE."""
                t_ = xb_pool.tile([128, 2, C], f16, name=f"xb_{name}{ch}",
                                  tag="xb")
                nc.vector.tensor_copy(t_[:], x32[(name, ch)][:])
                xb[(name, ch)] = t_

            def T_t(name, t):
                """PE-transpose x rows t*128..t*128+128 into xT[name]."""
                src = xb[(name, t // 2)]
                ti = t % 2
                pst = ps_p.tile([128, 4, 128], f16, tag="pp",
                                name=f"pst_{name}_{t}")
                for cc in range(4):
                    nc.tensor.transpose(
                        pst[:, cc, :],
                        src[:, ti, cc * 128:(cc + 1) * 128],
                        ident_f16[:])
                nc.vector.tensor_copy(
                    xT[name][:, :, t * 128:(t + 1) * 128], pst[:])

            def projqk(name, dt, qhs=(0, 1)):
                """Q/K projection d-chunk dt for the given q-halves.

                stationary: fp16 W slice, moving: fp16 xT.
                """
                tgt = QT if name == "q" else KT
                psqs = {}
                for qh in qhs:
                    psqs[qh] = ps_p.tile([128, 512], f32, tag="pp",
                                         name=f"psq_{name}{dt}_{qh}")
                for cc in range(4):
                    w_slice = w16[name][:, cc, dt * 128:(dt + 1) * 128]
                    for qh in qhs:
                        nc.tensor.matmul(
                            psqs[qh][:],
                            w_slice,
                            xT[name][:, cc, qh * 512:(qh + 1) * 512],
                            start=(cc == 0), stop=(cc == 3))
                for qh in qhs:
                    nc.vector.tensor_scalar_add(
                        tgt[:, dt, qh * 512:(qh + 1) * 512], psqs[qh][:],
                        b_sb[name][:, dt:dt + 1])

            def projv(st):
                """V projection s-chunk st: stationary xT_v fp16 (FWL),
                moving W fp16; ones-column bias matmul; evac to V_aug."""
                psv = ps_p.tile([128, 512], f32, tag="pp", name=f"psv_{st}")
                for cc in range(4):
                    nc.tensor.matmul(
                        psv[:],
                        xT["v"][:, cc, st * 128:(st + 1) * 128],
                        w16["v"][:, cc, :],
                        start=(cc == 0), stop=False)
                nc.tensor.matmul(
                    psv[:], ones_16[0:1, 0:128], bv_16[0:1, :],
                    start=False, stop=True)
                nc.vector.tensor_copy(
                    V[:, st, :, 0:HD],
                    psv[:].rearrange("p (h e) -> p h e", h=HEADS))

            # ---------------- filler queue ----------------
            fillers = []

            def fill(n=1):
                for _ in range(n):
                    if fillers:
                        fillers.pop(0)()

            def drain_fillers():
                while fillers:
                    fillers.pop(0)()

            # ---------------- attention ----------------
            # Deferred tail (attV kt=7 + evacuation) per sub-block, flushed
            # inside the next sub-block after its first exp.
            pend = [None]

            def scores_exp(hp, qh, kt):
                heads = (2 * hp, 2 * hp + 1)
                pss = ps_s.tile([128, 2, 512], f32, tag="pss",
                                name=f"pss_{hp}_{qh}_{kt}")
                for i, h in enumerate(heads):
                    po = (h % 2) * HD
                    nc.tensor.matmul(
                        pss[:, i, :],
                        KT[po:po + HD, hp, kt * 128:(kt + 1) * 128],
                        QT[po:po + HD, hp, qh * 512:(qh + 1) * 512],
                        start=True, stop=True)
                attT = att_pool.tile([128, 2, 512], f16, tag="attT",
                                     name=f"attT_{hp}_{qh}_{kt}")
                nc.scalar.activation(attT[:], pss[:], Exp, scale=0.125)
                return attT

            def attention(hp):
                heads = (2 * hp, 2 * hp + 1)
                for qh in range(2):
                    atts = []
                    atts.append(scores_exp(hp, qh, 0))
                    if pend[0] is not None:
                        pend[0]()
                        pend[0] = None
                    pso = {}
                    for h in heads:
                        pso[h] = ps_o.tile([128, 512], f32,
                                           name=f"pso{h}_{qh}", tag="pso")
                    # software pipeline: attv(kt-1) emitted after scores(kt)
                    # so the PE never waits on the exp(kt) it just fed.
                    for kt in range(1, 8):
                        atts.append(scores_exp(hp, qh, kt))
                        fill(1)
                        for i, h in enumerate(heads):
                            nc.tensor.matmul(
                                pso[h][:],
                                V[:, kt - 1, h, :],
                                atts[kt - 1][:, i, :],
                                start=(kt == 1), stop=False)

                    att7 = atts[7]

                    def tail(heads=heads, pso=pso, att7=att7, qh=qh):
                        for i, h in enumerate(heads):
                            nc.tensor.matmul(
                                pso[h][:], V[:, 7, h, :], att7[:, i, :],
                                start=False, stop=True)
                        for h in heads:
                            oT = ot_pool.tile([HD + 1, 512], f32, tag="oT")
                            nc.vector.tensor_copy(oT[:], pso[h][0:HD + 1, :])
                            pbt = ps_o.tile([128, 4, HD + 1], f32, tag="pso",
                                            name=f"pbt{h}_{qh}")
                            for qs in range(4):
                                nc.tensor.transpose(
                                    pbt[:, qs, :],
                                    oT[:, qs * 128:(qs + 1) * 128],
                                    ident_f32[0:HD + 1, 0:HD + 1])
                            rec = ot_pool.tile([128, 4], f32, tag="rec")
                            nc.vector.reciprocal(rec[:], pbt[:, :, HD])
                            for qs in range(4):
                                qt = qh * 4 + qs
                                nc.vector.tensor_scalar_mul(
                                    o_stage[:, qt, h * HD:(h + 1) * HD],
                                    pbt[:, qs, 0:HD],
                                    rec[:, qs:qs + 1])

                    pend[0] = tail

            # ---------------- prefix ----------------
            for ch in range(4):
                cv("k", ch)
                T_t("k", 2 * ch)
                T_t("k", 2 * ch + 1)
            projqk("k", 0)
            for ch in range(2):
                cv("q", ch)
                T_t("q", 2 * ch)
                T_t("q", 2 * ch + 1)
            projqk("q", 0, qhs=(0,))
            cv("v", 0)
            T_t("v", 0)
            projv(0)
            T_t("v", 1)
            projv(1)

            # ---------------- filler schedule ----------------
            def u(*fns):
                def unit():
                    for f in fns:
                        f()
                return unit

            fillers.extend([
                # during (0, qh0): keep V st chain >= 1 ahead of attv, plus
                # finish xT_q and QT dt0 qh1 (needed at (0, qh1) kt0).
                u(lambda: cv("v", 1), lambda: T_t("v", 2), lambda: projv(2)),
                u(lambda: T_t("v", 3), lambda: projv(3),
                  lambda: cv("q", 2), lambda: T_t("q", 4), lambda: T_t("q", 5)),
                u(lambda: cv("v", 2), lambda: T_t("v", 4), lambda: projv(4),
                  lambda: cv("q", 3), lambda: T_t("q", 6), lambda: T_t("q", 7)),
                u(lambda: T_t("v", 5), lambda: projv(5),
                  lambda: projqk("q", 0, qhs=(1,))),
                u(lambda: cv("v", 3), lambda: T_t("v", 6), lambda: projv(6)),
                u(lambda: T_t("v", 7), lambda: projv(7)),
                u(lambda: projqk("k", 1)),
                # during (0, qh1)
                u(lambda: projqk("q", 1)),
                u(lambda: projqk("k", 2)),
                u(lambda: projqk("q", 2)),
                u(lambda: projqk("k", 3)),
                # during (1, qh0)
                u(lambda: projqk("q", 3)),
            ])

            out_r = out_d[:].rearrange("(t p) d -> p t d", p=128)

            attention(0)
            attention(1)
            # hp0 tails flushed inside attention(1)'s first sub-block
            nc.sync.dma_start(out_r[:, :, 0:128], o_stage[:, :, 0:128])
            attention(2)
            nc.sync.dma_start(out_r[:, :, 128:256], o_stage[:, :, 128:256])
            attention(3)
            nc.sync.dma_start(out_r[:, :, 256:384], o_stage[:, :, 256:384])
            pend[0]()
            pend[0] = None
            drain_fillers()
            nc.sync.dma_start(out_r[:, :, 384:512], o_stage[:, :, 384:512])

    nc.compile()
    return nc


_NC = None


def _get_nc():
    global _NC
    if _NC is None:
        _NC = build_nc()
    return _NC


def _make_in_maps(inputs):
    in_maps = []
    for b in range(B):
        m = {
            "q_in": np.ascontiguousarray(np.asarray(inputs["q_in"])[b].reshape(S, C)),
            "k_in": np.ascontiguousarray(np.asarray(inputs["k_in"])[b].reshape(S, C)),
            "v_in": np.ascontiguousarray(np.asarray(inputs["v_in"])[b].reshape(S, C)),
            "Wq": np.asarray(inputs["Wq"]), "bq": np.asarray(inputs["bq"]),
            "Wk": np.asarray(inputs["Wk"]), "bk": np.asarray(inputs["bk"]),
            "Wv": np.asarray(inputs["Wv"]), "bv": np.asarray(inputs["bv"]),
        }
        in_maps.append(m)
    return in_maps


def kernel(**inputs):
    nc = _get_nc()
    res = run_bass_kernel_spmd(nc, _make_in_maps(inputs), list(range(N_CORES)))
    out = np.stack([res.results[i]["out"] for i in range(B)])
    return out.reshape(B, HS, WS, D).astype(np.float32)


if __name__ == "__main__":
    rng = np.random.default_rng(0)
    ins = {
        "q_in": rng.standard_normal((B, HS, WS, C), dtype=np.float32),
        "k_in": rng.standard_normal((B, HS, WS, C), dtype=np.float32),
        "v_in": rng.standard_normal((B, HS, WS, C), dtype=np.float32),
        "Wq": (rng.standard_normal((C, D)) / np.sqrt(C)).astype(np.float32),
        "Wk": (rng.standard_normal((C, D)) / np.sqrt(C)).astype(np.float32),
        "Wv": (rng.standard_normal((C, D)) / np.sqrt(C)).astype(np.float32),
        "bq": np.zeros(D, np.float32),
        "bk": np.zeros(D, np.float32),
        "bv": np.zeros(D, np.float32),
    }
    out = kernel(**ins)
    print("out shape:", out.shape, "finite:", np.isfinite(out).all())


# revision 8
# speedup vs baseline: 1.3177x; 1.1551x over previous
"""MultiHeadAttention Trainium2 Bass kernel (v2: interleaved pipeline).

Problem: B=8, H=W=32 (S=1024), C=512, 8 heads x 64 dim.
Sharding: data-parallel over batch, one batch element per NeuronCore (8 cores).

v2 design vs v1 (150us baseline):
- x converted to fp16 (DVE) so input PE-transposes run at 1 cycle/col
  (fp32 transposes are 2 cycles/col with a serial weight load).
- Projections and attention all in fp16 (10 mantissa bits): x/W converted
  once, Q/K use fp16 W stationary with fp16 xT moving; V uses fp16 xT
  stationary (FWL fast weight load) with fp16 W moving.
- V_aug stationary padded to 128 cols (64 d + ones + zeros) so attV weight
  loads use FWL and hide under the previous matmul.
- Single-phase emission: projections/transposes/evacuations are interleaved
  into the attention stream as filler so the PE never idles waiting on exp
  (ACT) and never cools (HAM throttle).
- Per-head-pair output DMAs issued as soon as each slice is evacuated.

Precision: fp16 operands, fp32 accumulation. Simulated end-to-end absmax
rel err ~1.3e-3 (threshold 2e-2).
"""
import sys

import numpy as np

if "/opt/trn_rl_repo" not in sys.path:
    sys.path.insert(0, "/opt/trn_rl_repo")

import concourse.bacc as bacc
import concourse.mybir as mybir
import concourse.tile as tile
from concourse import masks
from concourse.bass_utils import run_bass_kernel_spmd

B, HS, WS, C = 8, 32, 32, 512
S = HS * WS          # 1024
D = 512
HEADS = 8
HD = 64              # head dim
N_CORES = 8

f32 = mybir.dt.float32
f16 = mybir.dt.float16
bf16 = mybir.dt.bfloat16
Exp = mybir.ActivationFunctionType.Exp


def build_nc():
    nc = bacc.Bacc("TRN2", target_bir_lowering=False, debug=False,
                   num_devices=N_CORES)

    x_d = {}
    w_d = {}
    b_d = {}
    for name in ("q", "k", "v"):
        x_d[name] = nc.dram_tensor(f"{name}_in", [S, C], f32, kind="ExternalInput")
        w_d[name] = nc.dram_tensor(f"W{name}", [C, D], f32, kind="ExternalInput")
        b_d[name] = nc.dram_tensor(f"b{name}", [D], f32, kind="ExternalInput")
    out_d = nc.dram_tensor("out", [S, D], f32, kind="ExternalOutput")

    with tile.TileContext(nc) as tc:
        with (
            tc.tile_pool(name="const", bufs=1) as cpool,
            tc.tile_pool(name="x32", bufs=5) as x32_pool,
            tc.tile_pool(name="xb", bufs=3) as xb_pool,
            tc.tile_pool(name="xT", bufs=1) as xt_pool,
            tc.tile_pool(name="wbuf", bufs=1) as w_pool,
            tc.tile_pool(name="proj", bufs=1) as proj_pool,
            tc.tile_pool(name="att", bufs=3) as att_pool,
            tc.tile_pool(name="ot", bufs=4) as ot_pool,
            tc.tile_pool(name="ps_p", bufs=2, space="PSUM") as ps_p,
            tc.tile_pool(name="ps_s", bufs=2, space="PSUM") as ps_s,
            tc.tile_pool(name="ps_o", bufs=2, space="PSUM") as ps_o,
        ):
            # ---------------- constants ----------------
            ident_f16 = cpool.tile([128, 128], f16)
            masks.make_identity(nc, ident_f16[:])
            ident_f32 = cpool.tile([128, 128], f32)
            masks.make_identity(nc, ident_f32[:])
            ones_sb = cpool.tile([128, 512], f32)
            nc.gpsimd.memset(ones_sb[:], 1.0)
            ones_16 = cpool.tile([1, 512], f16)
            nc.gpsimd.tensor_copy(ones_16[:], ones_sb[0:1, :])

            # ---------------- persistent tensors ----------------
            QT = proj_pool.tile([128, 4, S], f16, name="QT")  # [d%128, d//128, s]
            KT = proj_pool.tile([128, 4, S], f16, name="KT")
            # V_aug padded: [s%128, s//128, head, 128]; col 64 = 1.0 (denom),
            # cols 65.. = 0 so the 128-wide stationary gets FWL.
            V = proj_pool.tile([128, 8, HEADS, 128], f16, name="V")
            zz = cpool.tile([128, 512], f16)
            nc.gpsimd.memset(zz[:], 0.0)
            for st8 in range(8):
                nc.gpsimd.tensor_copy(
                    V[:, st8, :, HD + 1:],
                    zz[:, 0:8 * 63].rearrange("p (a o) -> p a o", a=8))
            nc.gpsimd.tensor_copy(
                V[:, :, :, HD:HD + 1],
                ones_sb[:, 0:64].rearrange("p (a b o) -> p a b o", a=8, b=8))
            o_stage = proj_pool.tile([128, 8, D], f32, name="o_stage")

            # xT tiles (f16): [c%128, c//128, s]
            xT = {}
            for name in ("q", "k", "v"):
                xT[name] = xt_pool.tile([128, 4, S], f16, name=f"xT_{name}",
                                        tag=f"xT_{name}")

            # ---------------- input DMAs (priority order: x before W) -------
            x32 = {}   # (name, ch) -> tile
            def dma_x(name, ch):
                t_ = x32_pool.tile([128, 2, C], f32, name=f"x32_{name}{ch}",
                                   tag="x32")
                x_r = x_d[name][:].rearrange("(t p) c -> p t c", p=128)
                nc.sync.dma_start(t_[:], x_r[:, 2 * ch:2 * ch + 2, :])
                x32[(name, ch)] = t_

            w_sb = {}
            w16 = {}
            def dma_w(name):
                w_sb[name] = w_pool.tile([128, 4, D], f32, name=f"w_{name}",
                                         tag=f"w_{name}")
                nc.sync.dma_start(
                    w_sb[name][:],
                    w_d[name][:].rearrange("(cc p) d -> p cc d", p=128))
                # fp16 convert on the scalar engine (idle until first exp)
                w16[name] = w_pool.tile([128, 4, D], f16, name=f"w16_{name}",
                                        tag=f"w16_{name}")
                nc.scalar.copy(w16[name][:], w_sb[name][:])

            for ch in range(4):
                dma_x("k", ch)
            dma_w("k")
            dma_x("q", 0)
            dma_x("q", 1)
            dma_w("q")
            dma_x("v", 0)
            dma_x("v", 1)
            dma_w("v")
            b_sb = {}
            for name in ("k", "q"):
                b_sb[name] = w_pool.tile([128, 4], f32, name=f"b_{name}",
                                         tag=f"b_{name}")
                nc.sync.dma_start(
                    b_sb[name][:], b_d[name][:].rearrange("(dt p) -> p dt", p=128))
            bv_sb = w_pool.tile([1, D], f32, name="bv_sb", tag="bv_sb")
            nc.sync.dma_start(
                bv_sb[:], b_d["v"][:].rearrange("(o d) -> o d", o=1))
            bv_16 = w_pool.tile([1, D], f16, name="bv_16", tag="bv_16")
            nc.scalar.copy(bv_16[:], bv_sb[:])
            dma_x("q", 2)
            dma_x("q", 3)
            dma_x("v", 2)
            dma_x("v", 3)

            # ---------------- emission helpers ----------------
            xb = {}

            def cv(name, ch):
                """Convert x chunk to fp16 on DVE."""
                t_ = xb_pool.tile([128, 2, C], f16, name=f"xb_{name}{ch}",
                                  tag="xb")
                nc.vector.tensor_copy(t_[:], x32[(name, ch)][:])
                xb[(name, ch)] = t_

            def T_t(name, t):
                """PE-transpose x rows t*128..t*128+128 into xT[name]."""
                src = xb[(name, t // 2)]
                ti = t % 2
                pst = ps_p.tile([128, 4, 128], f16, tag="pp",
                                name=f"pst_{name}_{t}")
                for cc in range(4):
                    nc.tensor.transpose(
                        pst[:, cc, :],
                        src[:, ti, cc * 128:(cc + 1) * 128],
                        ident_f16[:])
                nc.vector.tensor_copy(
                    xT[name][:, :, t * 128:(t + 1) * 128], pst[:])

            def projqk(name, dt, qhs=(0, 1)):
                """Q/K projection d-chunk dt for the given q-halves.

                stationary: fp16 W slice, moving: fp16 xT.
                """
                tgt = QT if name == "q" else KT
                psqs = {}
                for qh in qhs:
                    psqs[qh] = ps_p.tile([128, 512], f32, tag="pp",
                                         name=f"psq_{name}{dt}_{qh}")
                for cc in range(4):
                    w_slice = w16[name][:, cc, dt * 128:(dt + 1) * 128]
                    for qh in qhs:
                        nc.tensor.matmul(
                            psqs[qh][:],
                            w_slice,
                            xT[name][:, cc, qh * 512:(qh + 1) * 512],
                            start=(cc == 0), stop=(cc == 3))
                for qh in qhs:
                    nc.vector.tensor_scalar_add(
                        tgt[:, dt, qh * 512:(qh + 1) * 512], psqs[qh][:],
                        b_sb[name][:, dt:dt + 1])

            def projv(st):
                """V projection s-chunk st: stationary xT_v fp16 (FWL),
                moving W fp16; ones-column bias matmul; evac to V_aug."""
                psv = ps_p.tile([128, 512], f32, tag="pp", name=f"psv_{st}")
                for cc in range(4):
                    nc.tensor.matmul(
                        psv[:],
                        xT["v"][:, cc, st * 128:(st + 1) * 128],
                        w16["v"][:, cc, :],
                        start=(cc == 0), stop=False)
                nc.tensor.matmul(
                    psv[:], ones_16[0:1, 0:128], bv_16[0:1, :],
                    start=False, stop=True)
                nc.vector.tensor_copy(
                    V[:, st, :, 0:HD],
                    psv[:].rearrange("p (h e) -> p h e", h=HEADS))

            # ---------------- filler queue ----------------
            fillers = []

            def fill(n=1):
                for _ in range(n):
                    if fillers:
                        fillers.pop(0)()

            def drain_fillers():
                while fillers:
                    fillers.pop(0)()

            # ---------------- attention ----------------
            # Deferred tail (attV kt=7 + evacuation) per sub-block, flushed
            # inside the next sub-block after its first exp.
            pend = [None]

            def scores_exp(hp, qh, kt):
                heads = (2 * hp, 2 * hp + 1)
                pss = ps_s.tile([128, 2, 512], f32, tag="pss",
                                name=f"pss_{hp}_{qh}_{kt}")
                for i, h in enumerate(heads):
                    po = (h % 2) * HD
                    nc.tensor.matmul(
                        pss[:, i, :],
                        KT[po:po + HD, hp, kt * 128:(kt + 1) * 128],
                        QT[po:po + HD, hp, qh * 512:(qh + 1) * 512],
                        start=True, stop=True)
                attT = att_pool.tile([128, 2, 512], bf16, tag="attT",
                                     name=f"attT_{hp}_{qh}_{kt}")
                nc.scalar.activation(attT[:], pss[:], Exp, scale=0.125)
                return attT

            def attention(hp):
                heads = (2 * hp, 2 * hp + 1)
                for qh in range(2):
                    atts = []
                    atts.append(scores_exp(hp, qh, 0))
                    if pend[0] is not None:
                        pend[0]()
                        pend[0] = None
                    pso = {}
                    for h in heads:
                        pso[h] = ps_o.tile([128, 512], f32,
                                           name=f"pso{h}_{qh}", tag="pso")
                    # software pipeline: attv(kt-1) emitted after scores(kt)
                    # so the PE never waits on the exp(kt) it just fed.
                    for kt in range(1, 8):
                        atts.append(scores_exp(hp, qh, kt))
                        fill(1)
                        for i, h in enumerate(heads):
                            nc.tensor.matmul(
                                pso[h][:],
                                V[:, kt - 1, h, :],
                                atts[kt - 1][:, i, :],
                                start=(kt == 1), stop=False)

                    att7 = atts[7]

                    def tail(heads=heads, pso=pso, att7=att7, qh=qh):
                        for i, h in enumerate(heads):
                            nc.tensor.matmul(
                                pso[h][:], V[:, 7, h, :], att7[:, i, :],
                                start=False, stop=True)
                        for h in heads:
                            oT = ot_pool.tile([HD + 1, 512], f32, tag="oT")
                            nc.vector.tensor_copy(oT[:], pso[h][0:HD + 1, :])
                            pbt = ps_o.tile([128, 4, HD + 1], f32, tag="pso",
                                            name=f"pbt{h}_{qh}")
                            for qs in range(4):
                                nc.tensor.transpose(
                                    pbt[:, qs, :],
                                    oT[:, qs * 128:(qs + 1) * 128],
                                    ident_f32[0:HD + 1, 0:HD + 1])
                            rec = ot_pool.tile([128, 4], f32, tag="rec")
                            nc.vector.reciprocal(rec[:], pbt[:, :, HD])
                            for qs in range(4):
                                qt = qh * 4 + qs
                                nc.vector.tensor_scalar_mul(
                                    o_stage[:, qt, h * HD:(h + 1) * HD],
                                    pbt[:, qs, 0:HD],
                                    rec[:, qs:qs + 1])

                    pend[0] = tail

            # ---------------- prefix ----------------
            for ch in range(4):
                cv("k", ch)
                T_t("k", 2 * ch)
                T_t("k", 2 * ch + 1)
            projqk("k", 0)
            for ch in range(2):
                cv("q", ch)
                T_t("q", 2 * ch)
                T_t("q", 2 * ch + 1)
            projqk("q", 0, qhs=(0,))
            cv("v", 0)
            T_t("v", 0)
            projv(0)
            T_t("v", 1)
            projv(1)

            # ---------------- filler schedule ----------------
            def u(*fns):
                def unit():
                    for f in fns:
                        f()
                return unit

            fillers.extend([
                # during (0, qh0): keep V st chain >= 1 ahead of attv, plus
                # finish xT_q and QT dt0 qh1 (needed at (0, qh1) kt0).
                u(lambda: cv("v", 1), lambda: T_t("v", 2), lambda: projv(2)),
                u(lambda: T_t("v", 3), lambda: projv(3),
                  lambda: cv("q", 2), lambda: T_t("q", 4), lambda: T_t("q", 5)),
                u(lambda: cv("v", 2), lambda: T_t("v", 4), lambda: projv(4),
                  lambda: cv("q", 3), lambda: T_t("q", 6), lambda: T_t("q", 7)),
                u(lambda: T_t("v", 5), lambda: projv(5),
                  lambda: projqk("q", 0, qhs=(1,))),
                u(lambda: cv("v", 3), lambda: T_t("v", 6), lambda: projv(6)),
                u(lambda: T_t("v", 7), lambda: projv(7)),
                u(lambda: projqk("k", 1)),
                # during (0, qh1)
                u(lambda: projqk("q", 1)),
                u(lambda: projqk("k", 2)),
                u(lambda: projqk("q", 2)),
                u(lambda: projqk("k", 3)),
                # during (1, qh0)
                u(lambda: projqk("q", 3)),
            ])

            out_r = out_d[:].rearrange("(t p) d -> p t d", p=128)

            attention(0)
            attention(1)
            # hp0 tails flushed inside attention(1)'s first sub-block
            nc.sync.dma_start(out_r[:, :, 0:128], o_stage[:, :, 0:128])
            attention(2)
            nc.sync.dma_start(out_r[:, :, 128:256], o_stage[:, :, 128:256])
            attention(3)
            nc.sync.dma_start(out_r[:, :, 256:384], o_stage[:, :, 256:384])
            pend[0]()
            pend[0] = None
            drain_fillers()
            nc.sync.dma_start(out_r[:, :, 384:512], o_stage[:, :, 384:512])

    nc.compile()
    return nc


_NC = None


def _get_nc():
    global _NC
    if _NC is None:
        _NC = build_nc()
    return _NC


def _make_in_maps(inputs):
    in_maps = []
    for b in range(B):
        m = {
            "q_in": np.ascontiguousarray(np.asarray(inputs["q_in"])[b].reshape(S, C)),
            "k_in": np.ascontiguousarray(np.asarray(inputs["k_in"])[b].reshape(S, C)),
            "v_in": np.ascontiguousarray(np.asarray(inputs["v_in"])[b].reshape(S, C)),
            "Wq": np.asarray(inputs["Wq"]), "bq": np.asarray(inputs["bq"]),
            "Wk": np.asarray(inputs["Wk"]), "bk": np.asarray(inputs["bk"]),
            "Wv": np.asarray(inputs["Wv"]), "bv": np.asarray(inputs["bv"]),
        }
        in_maps.append(m)
    return in_maps


def kernel(**inputs):
    nc = _get_nc()
    res = run_bass_kernel_spmd(nc, _make_in_maps(inputs), list(range(N_CORES)))
    out = np.stack([res.results[i]["out"] for i in range(B)])
    return out.reshape(B, HS, WS, D).astype(np.float32)


if __name__ == "__main__":
    rng = np.random.default_rng(0)
    ins = {
        "q_in": rng.standard_normal((B, HS, WS, C), dtype=np.float32),
        "k_in": rng.standard_normal((B, HS, WS, C), dtype=np.float32),
        "v_in": rng.standard_normal((B, HS, WS, C), dtype=np.float32),
        "Wq": (rng.standard_normal((C, D)) / np.sqrt(C)).astype(np.float32),
        "Wk": (rng.standard_normal((C, D)) / np.sqrt(C)).astype(np.float32),
        "Wv": (rng.standard_normal((C, D)) / np.sqrt(C)).astype(np.float32),
        "bq": np.zeros(D, np.float32),
        "bk": np.zeros(D, np.float32),
        "bv": np.zeros(D, np.float32),
    }
    out = kernel(**ins)
    print("out shape:", out.shape, "finite:", np.isfinite(out).all())


# revision 9
# speedup vs baseline: 1.3310x; 1.0101x over previous
"""MultiHeadAttention Trainium2 Bass kernel (v2: interleaved pipeline).

Problem: B=8, H=W=32 (S=1024), C=512, 8 heads x 64 dim.
Sharding: data-parallel over batch, one batch element per NeuronCore (8 cores).

v2 design vs v1 (150us baseline):
- x converted to fp16 (DVE) so input PE-transposes run at 1 cycle/col
  (fp32 transposes are 2 cycles/col with a serial weight load).
- Projections and attention all in fp16 (10 mantissa bits): x/W converted
  once, Q/K use fp16 W stationary with fp16 xT moving; V uses fp16 xT
  stationary (FWL fast weight load) with fp16 W moving.
- V_aug stationary padded to 128 cols (64 d + ones + zeros) so attV weight
  loads use FWL and hide under the previous matmul.
- Single-phase emission: projections/transposes/evacuations are interleaved
  into the attention stream as filler so the PE never idles waiting on exp
  (ACT) and never cools (HAM throttle).
- Per-head-pair output DMAs issued as soon as each slice is evacuated.

Precision: fp16 operands, fp32 accumulation. Simulated end-to-end absmax
rel err ~1.3e-3 (threshold 2e-2).
"""
import sys

import numpy as np

if "/opt/trn_rl_repo" not in sys.path:
    sys.path.insert(0, "/opt/trn_rl_repo")

import concourse.bacc as bacc
import concourse.mybir as mybir
import concourse.tile as tile
from concourse import masks
from concourse.bass_utils import run_bass_kernel_spmd

B, HS, WS, C = 8, 32, 32, 512
S = HS * WS          # 1024
D = 512
HEADS = 8
HD = 64              # head dim
N_CORES = 8

f32 = mybir.dt.float32
f16 = mybir.dt.float16
bf16 = mybir.dt.bfloat16
Exp = mybir.ActivationFunctionType.Exp


def build_nc():
    nc = bacc.Bacc("TRN2", target_bir_lowering=False, debug=False,
                   num_devices=N_CORES)

    x_d = {}
    w_d = {}
    b_d = {}
    for name in ("q", "k", "v"):
        x_d[name] = nc.dram_tensor(f"{name}_in", [S, C], f32, kind="ExternalInput")
        w_d[name] = nc.dram_tensor(f"W{name}", [C, D], f32, kind="ExternalInput")
        b_d[name] = nc.dram_tensor(f"b{name}", [D], f32, kind="ExternalInput")
    out_d = nc.dram_tensor("out", [S, D], f32, kind="ExternalOutput")

    with tile.TileContext(nc) as tc:
        with (
            tc.tile_pool(name="const", bufs=1) as cpool,
            tc.tile_pool(name="x32", bufs=5) as x32_pool,
            tc.tile_pool(name="xb", bufs=3) as xb_pool,
            tc.tile_pool(name="xT", bufs=1) as xt_pool,
            tc.tile_pool(name="wbuf", bufs=1) as w_pool,
            tc.tile_pool(name="proj", bufs=1) as proj_pool,
            tc.tile_pool(name="att", bufs=3) as att_pool,
            tc.tile_pool(name="ot", bufs=4) as ot_pool,
            tc.tile_pool(name="ps_p", bufs=2, space="PSUM") as ps_p,
            tc.tile_pool(name="ps_s", bufs=2, space="PSUM") as ps_s,
            tc.tile_pool(name="ps_o", bufs=2, space="PSUM") as ps_o,
        ):
            # ---------------- constants ----------------
            ident_f16 = cpool.tile([128, 128], f16)
            masks.make_identity(nc, ident_f16[:])
            ident_f32 = cpool.tile([128, 128], f32)
            masks.make_identity(nc, ident_f32[:])
            ones_sb = cpool.tile([128, 512], f32)
            nc.vector.memset(ones_sb[:], 1.0)
            ones_16 = cpool.tile([1, 512], f16)
            nc.vector.tensor_copy(ones_16[:], ones_sb[0:1, :])

            # ---------------- persistent tensors ----------------
            QT = proj_pool.tile([128, 4, S], f16, name="QT")  # [d%128, d//128, s]
            KT = proj_pool.tile([128, 4, S], f16, name="KT")
            # V_aug padded: [s%128, s//128, head, 128]; col 64 = 1.0 (denom),
            # cols 65.. = 0 so the 128-wide stationary gets FWL.
            V = proj_pool.tile([128, 8, HEADS, 128], f16, name="V")
            zz = cpool.tile([128, 512], f16)
            nc.vector.memset(zz[:], 0.0)
            for st8 in range(8):
                nc.vector.tensor_copy(
                    V[:, st8, :, HD + 1:],
                    zz[:, 0:8 * 63].rearrange("p (a o) -> p a o", a=8))
            nc.vector.tensor_copy(
                V[:, :, :, HD:HD + 1],
                ones_sb[:, 0:64].rearrange("p (a b o) -> p a b o", a=8, b=8))
            o_stage = proj_pool.tile([128, 8, D], f32, name="o_stage")

            # xT tiles (f16): [c%128, c//128, s]
            xT = {}
            for name in ("q", "k", "v"):
                xT[name] = xt_pool.tile([128, 4, S], f16, name=f"xT_{name}",
                                        tag=f"xT_{name}")

            # ---------------- input DMAs (priority order: x before W) -------
            x32 = {}   # (name, ch) -> tile
            def dma_x(name, ch):
                t_ = x32_pool.tile([128, 2, C], f32, name=f"x32_{name}{ch}",
                                   tag="x32")
                x_r = x_d[name][:].rearrange("(t p) c -> p t c", p=128)
                nc.sync.dma_start(t_[:], x_r[:, 2 * ch:2 * ch + 2, :])
                x32[(name, ch)] = t_

            w_sb = {}
            w16 = {}
            def dma_w(name):
                w_sb[name] = w_pool.tile([128, 4, D], f32, name=f"w_{name}",
                                         tag=f"w_{name}")
                nc.sync.dma_start(
                    w_sb[name][:],
                    w_d[name][:].rearrange("(cc p) d -> p cc d", p=128))
                # fp16 convert on the scalar engine (idle until first exp)
                w16[name] = w_pool.tile([128, 4, D], f16, name=f"w16_{name}",
                                        tag=f"w16_{name}")
                nc.scalar.copy(w16[name][:], w_sb[name][:])

            b_sb = {}
            for name in ("k", "q"):
                b_sb[name] = w_pool.tile([128, 4], f32, name=f"b_{name}",
                                         tag=f"b_{name}")
                nc.sync.dma_start(
                    b_sb[name][:], b_d[name][:].rearrange("(dt p) -> p dt", p=128))
            bv_sb = w_pool.tile([1, D], f32, name="bv_sb", tag="bv_sb")
            nc.sync.dma_start(
                bv_sb[:], b_d["v"][:].rearrange("(o d) -> o d", o=1))
            bv_16 = w_pool.tile([1, D], f16, name="bv_16", tag="bv_16")
            nc.scalar.copy(bv_16[:], bv_sb[:])
            for ch in range(4):
                dma_x("k", ch)
            dma_w("k")
            dma_x("q", 0)
            dma_x("q", 1)
            dma_w("q")
            dma_x("v", 0)
            dma_x("v", 1)
            dma_w("v")
            dma_x("v", 2)
            dma_x("q", 2)
            dma_x("v", 3)
            dma_x("q", 3)

            # ---------------- emission helpers ----------------
            xb = {}

            def cv(name, ch):
                """Convert x chunk to fp16 on DVE."""
                t_ = xb_pool.tile([128, 2, C], f16, name=f"xb_{name}{ch}",
                                  tag="xb")
                nc.vector.tensor_copy(t_[:], x32[(name, ch)][:])
                xb[(name, ch)] = t_

            def T_t(name, t):
                """PE-transpose x rows t*128..t*128+128 into xT[name]."""
                src = xb[(name, t // 2)]
                ti = t % 2
                pst = ps_p.tile([128, 4, 128], f16, tag="pp",
                                name=f"pst_{name}_{t}")
                for cc in range(4):
                    nc.tensor.transpose(
                        pst[:, cc, :],
                        src[:, ti, cc * 128:(cc + 1) * 128],
                        ident_f16[:])
                nc.vector.tensor_copy(
                    xT[name][:, :, t * 128:(t + 1) * 128], pst[:])

            def projqk(name, dt, qhs=(0, 1)):
                """Q/K projection d-chunk dt for the given q-halves.

                stationary: fp16 W slice, moving: fp16 xT.
                """
                tgt = QT if name == "q" else KT
                psqs = {}
                for qh in qhs:
                    psqs[qh] = ps_p.tile([128, 512], f32, tag="pp",
                                         name=f"psq_{name}{dt}_{qh}")
                for cc in range(4):
                    w_slice = w16[name][:, cc, dt * 128:(dt + 1) * 128]
                    for qh in qhs:
                        nc.tensor.matmul(
                            psqs[qh][:],
                            w_slice,
                            xT[name][:, cc, qh * 512:(qh + 1) * 512],
                            start=(cc == 0), stop=(cc == 3))
                for qh in qhs:
                    nc.vector.tensor_scalar_add(
                        tgt[:, dt, qh * 512:(qh + 1) * 512], psqs[qh][:],
                        b_sb[name][:, dt:dt + 1])

            def projv(st):
                """V projection s-chunk st: stationary xT_v fp16 (FWL),
                moving W fp16; ones-column bias matmul; evac to V_aug."""
                psv = ps_p.tile([128, 512], f32, tag="pp", name=f"psv_{st}")
                for cc in range(4):
                    nc.tensor.matmul(
                        psv[:],
                        xT["v"][:, cc, st * 128:(st + 1) * 128],
                        w16["v"][:, cc, :],
                        start=(cc == 0), stop=False)
                nc.tensor.matmul(
                    psv[:], ones_16[0:1, 0:128], bv_16[0:1, :],
                    start=False, stop=True)
                nc.vector.tensor_copy(
                    V[:, st, :, 0:HD],
                    psv[:].rearrange("p (h e) -> p h e", h=HEADS))

            # ---------------- filler queue ----------------
            fillers = []

            def fill(n=1):
                for _ in range(n):
                    if fillers:
                        fillers.pop(0)()

            def drain_fillers():
                while fillers:
                    fillers.pop(0)()

            # ---------------- attention ----------------
            # Deferred tail (attV kt=7 + evacuation) per sub-block, flushed
            # inside the next sub-block after its first exp.
            pend = [None]

            def scores_exp(hp, qh, kt):
                heads = (2 * hp, 2 * hp + 1)
                pss = ps_s.tile([128, 2, 512], f32, tag="pss",
                                name=f"pss_{hp}_{qh}_{kt}")
                for i, h in enumerate(heads):
                    po = (h % 2) * HD
                    nc.tensor.matmul(
                        pss[:, i, :],
                        KT[po:po + HD, hp, kt * 128:(kt + 1) * 128],
                        QT[po:po + HD, hp, qh * 512:(qh + 1) * 512],
                        start=True, stop=True)
                attT = att_pool.tile([128, 2, 512], bf16, tag="attT",
                                     name=f"attT_{hp}_{qh}_{kt}")
                nc.scalar.activation(attT[:], pss[:], Exp, scale=0.125)
                return attT

            def attention(hp):
                heads = (2 * hp, 2 * hp + 1)
                for qh in range(2):
                    atts = []
                    atts.append(scores_exp(hp, qh, 0))
                    if pend[0] is not None:
                        pend[0]()
                        pend[0] = None
                    pso = {}
                    for h in heads:
                        pso[h] = ps_o.tile([128, 512], f32,
                                           name=f"pso{h}_{qh}", tag="pso")
                    # software pipeline: attv(kt-1) emitted after scores(kt)
                    # so the PE never waits on the exp(kt) it just fed.
                    for kt in range(1, 8):
                        atts.append(scores_exp(hp, qh, kt))
                        fill(1)
                        for i, h in enumerate(heads):
                            nc.tensor.matmul(
                                pso[h][:],
                                V[:, kt - 1, h, :],
                                atts[kt - 1][:, i, :],
                                start=(kt == 1), stop=False)

                    att7 = atts[7]

                    def tail(heads=heads, pso=pso, att7=att7, qh=qh):
                        for i, h in enumerate(heads):
                            nc.tensor.matmul(
                                pso[h][:], V[:, 7, h, :], att7[:, i, :],
                                start=False, stop=True)
                        for h in heads:
                            oT = ot_pool.tile([HD + 1, 512], f32, tag="oT")
                            nc.vector.tensor_copy(oT[:], pso[h][0:HD + 1, :])
                            pbt = ps_o.tile([128, 4, HD + 1], f32, tag="pso",
                                            name=f"pbt{h}_{qh}")
                            for qs in range(4):
                                nc.tensor.transpose(
                                    pbt[:, qs, :],
                                    oT[:, qs * 128:(qs + 1) * 128],
                                    ident_f32[0:HD + 1, 0:HD + 1])
                            rec = ot_pool.tile([128, 4], f32, tag="rec")
                            nc.vector.reciprocal(rec[:], pbt[:, :, HD])
                            for qs in range(4):
                                qt = qh * 4 + qs
                                nc.vector.tensor_scalar_mul(
                                    o_stage[:, qt, h * HD:(h + 1) * HD],
                                    pbt[:, qs, 0:HD],
                                    rec[:, qs:qs + 1])

                    pend[0] = tail

            # ---------------- prefix ----------------
            for ch in range(4):
                cv("k", ch)
                T_t("k", 2 * ch)
                T_t("k", 2 * ch + 1)
            projqk("k", 0)
            for ch in range(2):
                cv("q", ch)
                T_t("q", 2 * ch)
                T_t("q", 2 * ch + 1)
            projqk("q", 0, qhs=(0,))
            cv("v", 0)
            T_t("v", 0)
            projv(0)
            T_t("v", 1)
            projv(1)

            # ---------------- filler schedule ----------------
            def u(*fns):
                def unit():
                    for f in fns:
                        f()
                return unit

            fillers.extend([
                # during (0, qh0): keep V st chain >= 1 ahead of attv, plus
                # finish xT_q and QT dt0 qh1 (needed at (0, qh1) kt0).
                u(lambda: cv("v", 1), lambda: T_t("v", 2), lambda: projv(2)),
                u(lambda: T_t("v", 3), lambda: projv(3),
                  lambda: cv("q", 2), lambda: T_t("q", 4), lambda: T_t("q", 5)),
                u(lambda: cv("v", 2), lambda: T_t("v", 4), lambda: projv(4),
                  lambda: cv("q", 3), lambda: T_t("q", 6), lambda: T_t("q", 7)),
                u(lambda: T_t("v", 5), lambda: projv(5),
                  lambda: projqk("q", 0, qhs=(1,))),
                u(lambda: cv("v", 3), lambda: T_t("v", 6), lambda: projv(6)),
                u(lambda: T_t("v", 7), lambda: projv(7)),
                u(lambda: projqk("k", 1)),
                # during (0, qh1)
                u(lambda: projqk("q", 1)),
                u(lambda: projqk("k", 2)),
                u(lambda: projqk("q", 2)),
                u(lambda: projqk("k", 3)),
                # during (1, qh0)
                u(lambda: projqk("q", 3)),
            ])

            out_r = out_d[:].rearrange("(t p) d -> p t d", p=128)

            attention(0)
            attention(1)
            # hp0 tails flushed inside attention(1)'s first sub-block
            nc.sync.dma_start(out_r[:, :, 0:128], o_stage[:, :, 0:128])
            attention(2)
            nc.sync.dma_start(out_r[:, :, 128:256], o_stage[:, :, 128:256])
            attention(3)
            nc.sync.dma_start(out_r[:, :, 256:384], o_stage[:, :, 256:384])
            pend[0]()
            pend[0] = None
            drain_fillers()
            nc.sync.dma_start(out_r[:, :, 384:512], o_stage[:, :, 384:512])

    nc.compile()
    return nc


_NC = None


def _get_nc():
    global _NC
    if _NC is None:
        _NC = build_nc()
    return _NC


def _make_in_maps(inputs):
    in_maps = []
    for b in range(B):
        m = {
            "q_in": np.ascontiguousarray(np.asarray(inputs["q_in"])[b].reshape(S, C)),
            "k_in": np.ascontiguousarray(np.asarray(inputs["k_in"])[b].reshape(S, C)),
            "v_in": np.ascontiguousarray(np.asarray(inputs["v_in"])[b].reshape(S, C)),
            "Wq": np.asarray(inputs["Wq"]), "bq": np.asarray(inputs["bq"]),
            "Wk": np.asarray(inputs["Wk"]), "bk": np.asarray(inputs["bk"]),
            "Wv": np.asarray(inputs["Wv"]), "bv": np.asarray(inputs["bv"]),
        }
        in_maps.append(m)
    return in_maps


def kernel(**inputs):
    nc = _get_nc()
    res = run_bass_kernel_spmd(nc, _make_in_maps(inputs), list(range(N_CORES)))
    out = np.stack([res.results[i]["out"] for i in range(B)])
    return out.reshape(B, HS, WS, D).astype(np.float32)


if __name__ == "__main__":
    rng = np.random.default_rng(0)
    ins = {
        "q_in": rng.standard_normal((B, HS, WS, C), dtype=np.float32),
        "k_in": rng.standard_normal((B, HS, WS, C), dtype=np.float32),
        "v_in": rng.standard_normal((B, HS, WS, C), dtype=np.float32),
        "Wq": (rng.standard_normal((C, D)) / np.sqrt(C)).astype(np.float32),
        "Wk": (rng.standard_normal((C, D)) / np.sqrt(C)).astype(np.float32),
        "Wv": (rng.standard_normal((C, D)) / np.sqrt(C)).astype(np.float32),
        "bq": np.zeros(D, np.float32),
        "bk": np.zeros(D, np.float32),
        "bv": np.zeros(D, np.float32),
    }
    out = kernel(**ins)
    print("out shape:", out.shape, "finite:", np.isfinite(out).all())
